# revision 22
# baseline (speedup 1.0000x reference)
"""ChannelMerger v8: W-stationary phase 2, pre-normalized weights.

Phase 1 computes expT [c, o] (bf16) on PE/DVE/ACT/Pool, then the softmax
denominators as a single [1, O] psum row (ones as the STATIONARY operand:
a 1-column LDWEIGHTS is ~free, replacing 9 LDW+MM pairs with 3).  The
reciprocal row is partition-broadcast and folded into the weights
(wnorm = expT * recip, DVE), so phase-2 psum drains are plain copies with
no scalar-pointer dependency.  Phase 2 is W-stationary: stationary =
wnorm [csz, osz] reused across a 2048-column psum block (LDWEIGHTS
amortized 1:4 -- mandatory on this stack, where --enable-ldw-opt=false
makes 1:1 LDW:MM streams run at isolated-matmul latency), x streamed in
N=512 bf16 slices.

Other deltas vs the session baseline:
- Each 2048-wide psum drain is split into two 1024 halves, one on DVE and
  one on ACT, so the psum tile frees in about half the single-engine
  latency and both engines load evenly (PE micro-idle avoidance).
- Out stores stay on gpsimd SWDGE so the scalar HWDGE ring remains free
  for the next rep's phase-1 input loads (no rep-boundary PE bubble).

Host staging (untimed): x -> bf16 channel-major [C, B_LOC*T] per core;
heads -> bf16 packed headsT [128, 16, O]; layout -> [2, C]; out bf16
[O, B_LOC*T] -> host transpose/cast back to [B, O, T] f32.
"""

import sys

for _p in ("/opt/trn_rl_repo", "/root/.axon_site/_ro/trn_rl_repo"):
    if _p not in sys.path:
        sys.path.append(_p)

import numpy as np
import ml_dtypes

BF16 = ml_dtypes.bfloat16

B, C, T = 64, 270, 2000
O, D = 270, 2048
N_CORES = 8
B_LOC = B // N_CORES
NF = 32
MARGIN = 0.2
WIDTH = 1.4
KC = 8                           # ij chunks per half (8 * 128 = 1024 = D/2)
C_CHUNKS = [(0, 128), (128, 128), (256, 14)]
HALF = 8000                      # x/out processed in two 8000-column halves
PBLK = 2048                      # psum block width (4 banks); matmuls <=1024
BLT = B_LOC * T

_cache = {}


def _build(repeat=1):
    import concourse.tile as tile
    from concourse import bacc, mybir

    F32 = mybir.dt.float32
    BF = mybir.dt.bfloat16
    I32 = mybir.dt.int32
    ACT = mybir.ActivationFunctionType
    ALU = mybir.AluOpType
    TWO_PI = float(2.0 * np.pi)

    nc = bacc.Bacc("TRN2", target_bir_lowering=False, debug=False,
                   num_devices=N_CORES)

    xc_ap = nc.dram_tensor("xc", [C, BLT], BF, kind="ExternalInput").ap()
    lay_ap = nc.dram_tensor("lay2", [1, 2 * C], F32, kind="ExternalInput").ap()
    ht_ap = nc.dram_tensor("headsT", [128, 2 * KC, O], BF,
                           kind="ExternalInput").ap()
    tab_ap = nc.dram_tensor("tab", [128, 2 + KC], F32, kind="ExternalInput").ap()
    o01_ap = nc.dram_tensor("out01", [128, 2, BLT], BF,
                            kind="ExternalOutput").ap()
    o2_ap = nc.dram_tensor("out2", [14, BLT], BF,
                           kind="ExternalOutput").ap()

    with tile.TileContext(nc) as tc:
      with tc.tile_pool(name="const", bufs=1) as cpool, \
           tc.tile_pool(name="expTp", bufs=2) as epool, \
           tc.tile_pool(name="ph1", bufs=1) as wpool, \
           tc.tile_pool(name="xin", bufs=2) as xpool, \
           tc.tile_pool(name="oout", bufs=1) as opool, \
           tc.tile_pool(name="psum", bufs=2, space="PSUM") as mmp:
        for _rep in range(repeat):
            tab = cpool.tile([128, 2 + KC], F32, tag="tab")
            nc.scalar.dma_start(tab[:], tab_ap[:])
            headsT = cpool.tile([128, 2 * KC, O], BF, tag="headsT")
            nc.scalar.dma_start(headsT[:], ht_ap[:])
            lay = cpool.tile([1, 2 * C], F32, tag="lay")
            nc.scalar.dma_start(lay[:], lay_ap[:])

            u_row = cpool.tile([1, C], F32, tag="u_row")
            nc.vector.tensor_scalar(u_row[:], lay[:, 0:C], MARGIN, 1.0 / WIDTH,
                                    ALU.add, ALU.mult)
            v_row = cpool.tile([1, C], F32, tag="v_row")
            nc.vector.tensor_scalar(v_row[:], lay[:, C:2 * C], MARGIN,
                                    1.0 / WIDTH, ALU.add, ALU.mult)
            u_bc = cpool.tile([128, C], F32, tag="u_bc")
            nc.gpsimd.partition_broadcast(u_bc[:], u_row[:])
            v_bc = cpool.tile([128, C], F32, tag="v_bc")
            nc.gpsimd.partition_broadcast(v_bc[:], v_row[:])

            expT = [epool.tile([128, O], BF, tag=f"expT{i}", name=f"expT{i}")
                    for i in range(3)]

            # t2[p, c] = j(p) * v[c]
            t2 = wpool.tile([128, C], F32, tag="t2")
            nc.gpsimd.tensor_scalar(t2[:], v_bc[:], tab[:, 0:1], None, ALU.mult)
            # f_all[p, k, c] = i(k, p) * u[c] + t2[p, c]
            f_all = wpool.tile([128, KC, C], F32, tag="f_all")
            nc.vector.tensor_tensor(
                f_all[:],
                tab[:, 2:2 + KC].unsqueeze(2).broadcast_to([128, KC, C]),
                u_bc[:].unsqueeze(1).broadcast_to([128, KC, C]), ALU.mult)
            nc.vector.tensor_tensor(
                f_all[:], f_all[:],
                t2[:].unsqueeze(1).broadcast_to([128, KC, C]), ALU.add)
            # sin half: f - round(f) in [-.5,.5] on hw (f32->i32 rounds RNE)
            fi = wpool.tile([128, KC, C], I32, tag="fi")
            nc.vector.tensor_copy(fi[:], f_all[:])
            ff = wpool.tile([128, KC, C], F32, tag="ff")
            nc.gpsimd.tensor_copy(ff[:], fi[:])
            fs = wpool.tile([128, KC, C], F32, tag="fs")
            nc.vector.tensor_tensor(fs[:], f_all[:], ff[:], ALU.subtract)
            sin_all = wpool.tile([128, KC, C], BF, tag="sin_all")
            nc.scalar.activation(sin_all[:], fs[:], ACT.Sin, scale=TWO_PI)
            # cos half: frac(f + 0.25); reuse f_all/fi/ff/fs buffers
            g = wpool.tile([128, KC, C], F32, tag="f_all")
            nc.vector.tensor_scalar(g[:], fs[:], 0.25, None, ALU.add)
            gi = wpool.tile([128, KC, C], I32, tag="fi")
            nc.vector.tensor_copy(gi[:], g[:])
            gf = wpool.tile([128, KC, C], F32, tag="ff")
            nc.gpsimd.tensor_copy(gf[:], gi[:])
            gs = wpool.tile([128, KC, C], F32, tag="fs")
            nc.vector.tensor_tensor(gs[:], g[:], gf[:], ALU.subtract)
            cos_all = wpool.tile([128, KC, C], BF, tag="cos_all")
            nc.scalar.activation(cos_all[:], gs[:], ACT.Sin, scale=TWO_PI)

            ones = cpool.tile([128, 1], BF, tag="ones")
            nc.vector.memset(ones[:], 1.0)

            # scores / exp; psum slices come from the shared rotating pool
            for cc, (c0, csz) in enumerate(C_CHUNKS):
                ps = mmp.tile([128, PBLK], F32, tag="mm")
                for k in range(KC):
                    nc.tensor.matmul(ps[:csz, :O], cos_all[:, k, c0:c0 + csz],
                                     headsT[:, k, :], start=(k == 0),
                                     stop=False)
                for k in range(KC):
                    nc.tensor.matmul(ps[:csz, :O], sin_all[:, k, c0:c0 + csz],
                                     headsT[:, KC + k, :], start=False,
                                     stop=(k == KC - 1))
                nc.scalar.activation(expT[cc][:csz, :], ps[:csz, :O], ACT.Exp)
            # denominators as one [1, O] row: ones is the STATIONARY (1-column
            # LDWEIGHTS, ~free) and expT streams -- replaces 9 LDW+MM pairs
            # with 3.  The reciprocal row is broadcast over partitions and
            # folded into the weights, so psum drains become plain copies.
            ps = mmp.tile([128, PBLK], F32, tag="mm")
            for cc, (c0, csz) in enumerate(C_CHUNKS):
                nc.tensor.matmul(ps[0:1, 0:O], ones[:csz, 0:1],
                                 expT[cc][:csz, :],
                                 start=(cc == 0), stop=(cc == 2))
            recip_row = epool.tile([1, O], BF, tag="recip_row")
            with nc.allow_low_precision(reason="weights are bf16 anyway"):
                nc.vector.reciprocal(recip_row[:], ps[0:1, 0:O])
            recip_bc = epool.tile([128, O], BF, tag="recip_bc")
            nc.gpsimd.partition_broadcast(recip_bc[:], recip_row[:])
            wnorm = []
            for cc, (c0, csz) in enumerate(C_CHUNKS):
                wt = epool.tile([128, O], BF, tag=f"wn{cc}", name=f"wn{cc}")
                nc.vector.tensor_tensor(wt[:csz, :], expT[cc][:csz, :],
                                        recip_bc[:csz, :], ALU.mult)
                wnorm.append(wt)

            # ---- phase 2: two 8000-column halves, big DMAs ----
            dcount = 0
            for h in range(2):
                base = h * HALF
                xb = []
                for cc, (c0, csz) in enumerate(C_CHUNKS):
                    xt = xpool.tile([128, HALF], BF, tag=f"x{cc}",
                                    name=f"x{cc}")
                    nc.sync.dma_start(xt[:csz, :],
                                      xc_ap[c0:c0 + csz, base:base + HALF])
                    xb.append(xt)
                ot01 = opool.tile([128, 2, HALF], BF, tag="o01", name="o01")
                ot2 = opool.tile([128, HALF], BF, tag="o2", name="o2")
                for oc, (o0, osz) in enumerate(C_CHUNKS):
                    ot = ot2 if oc == 2 else ot01[:, oc, :]
                    for p0 in range(0, HALF, PBLK):
                        psz = min(PBLK, HALF - p0)
                        ph = mmp.tile([128, PBLK], F32, tag="mm")
                        for cc, (c0, csz) in enumerate(C_CHUNKS):
                            for s0 in range(0, psz, 512):
                                ssz = min(512, psz - s0)
                                nc.tensor.matmul(
                                    ph[:osz, s0:s0 + ssz],
                                    wnorm[cc][:csz, o0:o0 + osz],
                                    xb[cc][:csz, p0 + s0:p0 + s0 + ssz],
                                    start=(cc == 0), stop=(cc == 2))
                        # drain split across DVE+ACT so the psum tile frees
                        # in ~half the single-engine latency and both engines
                        # load evenly (alternate which engine takes the low
                        # half to decorrelate from other queue traffic)
                        hsz = psz // 2
                        lo = (ot[:osz, p0:p0 + hsz], ph[:osz, :hsz])
                        hi = (ot[:osz, p0 + hsz:p0 + psz], ph[:osz, hsz:psz])
                        a, b = (lo, hi) if dcount % 2 == 0 else (hi, lo)
                        dcount += 1
                        nc.vector.tensor_copy(a[0], a[1])
                        nc.scalar.activation(b[0], b[1], ACT.Copy)
                    # oc0+oc1 leave in ONE 4.1MB SWDGE store (out01 is laid
                    # out [p, j, t] so the combined tile maps directly); the
                    # 14-row remainder keeps its own small store
                    if oc == 1:
                        nc.gpsimd.dma_start(o01_ap[:, :, base:base + HALF],
                                            ot01[:, :, :])
                    elif oc == 2:
                        nc.gpsimd.dma_start(o2_ap[:, base:base + HALF],
                                            ot2[:14, :])

    nc.compile()
    return nc


def _tab_const():
    p = np.arange(128)
    cols = [(p & 31).astype(np.float32), np.ones(128, np.float32)]
    cols += [((k * 128 + p) >> 5).astype(np.float32) for k in range(KC)]
    return np.stack(cols, axis=1)


def _stage_heads(heads):
    hT = heads.T.astype(BF16)                     # [D, O]
    return np.ascontiguousarray(
        hT.reshape(2 * KC, 128, O).transpose(1, 0, 2))


def _stage_x(x_core):
    return np.ascontiguousarray(
        x_core.transpose(1, 0, 2).reshape(C, BLT).astype(BF16))


def get_nc(repeat=1):
    key = f"nc{repeat}"
    if key not in _cache:
        _cache[key] = _build(repeat)
    return _cache[key]


def make_in_maps(x, layout, heads):
    tab = _tab_const()
    ht = _stage_heads(heads.astype(np.float32))
    lay2 = np.ascontiguousarray(layout.astype(np.float32).T.reshape(1, 2 * C))
    return [
        {
            "xc": _stage_x(x[m * B_LOC:(m + 1) * B_LOC]),
            "lay2": lay2,
            "headsT": ht,
            "tab": tab,
        }
        for m in range(N_CORES)
    ]


def _core_rows(o01, o2):
    # out01 [128, 2, BLT]: row o = j*128 + p; out2 [14, BLT]: rows 256..269
    top = np.asarray(o01).transpose(1, 0, 2).reshape(256, BLT)
    full = np.concatenate([top, np.asarray(o2)], axis=0)
    return np.ascontiguousarray(
        full.reshape(O, B_LOC, T).transpose(1, 0, 2)).astype(np.float32)


def assemble_from_global(g01, g2):
    g01 = np.asarray(g01).reshape(N_CORES, 128, 2, BLT)
    g2 = np.asarray(g2).reshape(N_CORES, 14, BLT)
    return np.concatenate(
        [_core_rows(g01[m], g2[m]) for m in range(N_CORES)], axis=0)


def assemble_out(res_list):
    return np.concatenate(
        [_core_rows(res_list[m]["out01"], res_list[m]["out2"])
         for m in range(N_CORES)], axis=0)


def kernel(x, layout, heads):
    from concourse.bass_utils import run_bass_kernel_spmd

    assert x.shape == (B, C, T) and layout.shape == (C, 2)
    assert heads.shape == (O, D)
    nc = get_nc()
    in_maps = make_in_maps(x, layout, heads)
    res = run_bass_kernel_spmd(nc, in_maps, list(range(N_CORES)))
    return assemble_out(res.results)


# revision 23
# speedup vs baseline: 1.1965x; 1.1965x over previous
"""ChannelMerger v8: W-stationary phase 2, pre-normalized weights.

Phase 1 computes expT [c, o] (bf16) on PE/DVE/ACT/Pool, then the softmax
denominators as a single [1, O] psum row (ones as the STATIONARY operand:
a 1-column LDWEIGHTS is ~free, replacing 9 LDW+MM pairs with 3).  The
reciprocal row is partition-broadcast and folded into the weights
(wnorm = expT * recip, DVE), so phase-2 psum drains are plain copies with
no scalar-pointer dependency.  Phase 2 is W-stationary: stationary =
wnorm [csz, osz] reused across a 2048-column psum block (LDWEIGHTS
amortized 1:4 -- mandatory on this stack, where --enable-ldw-opt=false
makes 1:1 LDW:MM streams run at isolated-matmul latency), x streamed in
N=512 bf16 slices.

Other deltas vs the session baseline:
- Each 2048-wide psum drain is split into two 1024 halves, one on DVE and
  one on ACT, so the psum tile frees in about half the single-engine
  latency and both engines load evenly (PE micro-idle avoidance).
- Out stores stay on gpsimd SWDGE so the scalar HWDGE ring remains free
  for the next rep's phase-1 input loads (no rep-boundary PE bubble).

Host staging (untimed): x -> bf16 channel-major [C, B_LOC*T] per core;
heads -> bf16 packed headsT [128, 16, O]; layout -> [2, C]; out bf16
[O, B_LOC*T] -> host transpose/cast back to [B, O, T] f32.
"""

import sys

for _p in ("/opt/trn_rl_repo", "/root/.axon_site/_ro/trn_rl_repo"):
    if _p not in sys.path:
        sys.path.append(_p)

import numpy as np
import ml_dtypes

BF16 = ml_dtypes.bfloat16

B, C, T = 64, 270, 2000
O, D = 270, 2048
N_CORES = 8
B_LOC = B // N_CORES
NF = 32
MARGIN = 0.2
WIDTH = 1.4
KC = 8                           # ij chunks per half (8 * 128 = 1024 = D/2)
C_CHUNKS = [(0, 128), (128, 128), (256, 14)]
HALF = 8000                      # x/out processed in two 8000-column halves
PBLK = 2048                      # psum block width (4 banks); matmuls <=1024
BLT = B_LOC * T

_cache = {}


def _build(repeat=1):
    import concourse.tile as tile
    from concourse import bacc, mybir

    F32 = mybir.dt.float32
    BF = mybir.dt.bfloat16
    I32 = mybir.dt.int32
    ACT = mybir.ActivationFunctionType
    ALU = mybir.AluOpType
    TWO_PI = float(2.0 * np.pi)

    nc = bacc.Bacc("TRN2", target_bir_lowering=False, debug=False,
                   num_devices=N_CORES)

    xc_ap = nc.dram_tensor("xc", [C, BLT], BF, kind="ExternalInput").ap()
    lay_ap = nc.dram_tensor("lay2", [1, 2 * C], F32, kind="ExternalInput").ap()
    ht_ap = nc.dram_tensor("headsT", [128, 2 * KC, O], BF,
                           kind="ExternalInput").ap()
    tab_ap = nc.dram_tensor("tab", [128, 2 + KC], F32, kind="ExternalInput").ap()
    out_ap = nc.dram_tensor("out", [O, BLT], BF, kind="ExternalOutput").ap()

    with tile.TileContext(nc) as tc:
      with tc.tile_pool(name="const", bufs=1) as cpool, \
           tc.tile_pool(name="expTp", bufs=2) as epool, \
           tc.tile_pool(name="ph1", bufs=1) as wpool, \
           tc.tile_pool(name="xin", bufs=2) as xpool, \
           tc.tile_pool(name="oout", bufs=1) as opool, \
           tc.tile_pool(name="psum", bufs=2, space="PSUM") as mmp:
        for _rep in range(repeat):
            tab = cpool.tile([128, 2 + KC], F32, tag="tab")
            nc.scalar.dma_start(tab[:], tab_ap[:])
            headsT = cpool.tile([128, 2 * KC, O], BF, tag="headsT")
            nc.scalar.dma_start(headsT[:], ht_ap[:])
            lay = cpool.tile([1, 2 * C], F32, tag="lay")
            nc.scalar.dma_start(lay[:], lay_ap[:])

            u_row = cpool.tile([1, C], F32, tag="u_row")
            nc.vector.tensor_scalar(u_row[:], lay[:, 0:C], MARGIN, 1.0 / WIDTH,
                                    ALU.add, ALU.mult)
            v_row = cpool.tile([1, C], F32, tag="v_row")
            nc.vector.tensor_scalar(v_row[:], lay[:, C:2 * C], MARGIN,
                                    1.0 / WIDTH, ALU.add, ALU.mult)
            u_bc = cpool.tile([128, C], F32, tag="u_bc")
            nc.gpsimd.partition_broadcast(u_bc[:], u_row[:])
            v_bc = cpool.tile([128, C], F32, tag="v_bc")
            nc.gpsimd.partition_broadcast(v_bc[:], v_row[:])

            expT = [epool.tile([128, O], BF, tag=f"expT{i}", name=f"expT{i}")
                    for i in range(3)]

            # t2[p, c] = j(p) * v[c]
            t2 = wpool.tile([128, C], F32, tag="t2")
            nc.gpsimd.tensor_scalar(t2[:], v_bc[:], tab[:, 0:1], None, ALU.mult)
            # f_all[p, k, c] = i(k, p) * u[c] + t2[p, c]
            f_all = wpool.tile([128, KC, C], F32, tag="f_all")
            nc.vector.tensor_tensor(
                f_all[:],
                tab[:, 2:2 + KC].unsqueeze(2).broadcast_to([128, KC, C]),
                u_bc[:].unsqueeze(1).broadcast_to([128, KC, C]), ALU.mult)
            nc.vector.tensor_tensor(
                f_all[:], f_all[:],
                t2[:].unsqueeze(1).broadcast_to([128, KC, C]), ALU.add)
            # sin half: f - round(f) in [-.5,.5] on hw (f32->i32 rounds RNE)
            fi = wpool.tile([128, KC, C], I32, tag="fi")
            nc.vector.tensor_copy(fi[:], f_all[:])
            ff = wpool.tile([128, KC, C], F32, tag="ff")
            nc.gpsimd.tensor_copy(ff[:], fi[:])
            fs = wpool.tile([128, KC, C], F32, tag="fs")
            nc.vector.tensor_tensor(fs[:], f_all[:], ff[:], ALU.subtract)
            sin_all = wpool.tile([128, KC, C], BF, tag="sin_all")
            nc.scalar.activation(sin_all[:], fs[:], ACT.Sin, scale=TWO_PI)
            # cos half: frac(f + 0.25); reuse f_all/fi/ff/fs buffers
            g = wpool.tile([128, KC, C], F32, tag="f_all")
            nc.vector.tensor_scalar(g[:], fs[:], 0.25, None, ALU.add)
            gi = wpool.tile([128, KC, C], I32, tag="fi")
            nc.vector.tensor_copy(gi[:], g[:])
            gf = wpool.tile([128, KC, C], F32, tag="ff")
            nc.gpsimd.tensor_copy(gf[:], gi[:])
            gs = wpool.tile([128, KC, C], F32, tag="fs")
            nc.vector.tensor_tensor(gs[:], g[:], gf[:], ALU.subtract)
            cos_all = wpool.tile([128, KC, C], BF, tag="cos_all")
            nc.scalar.activation(cos_all[:], gs[:], ACT.Sin, scale=TWO_PI)

            ones = cpool.tile([128, 1], BF, tag="ones")
            nc.vector.memset(ones[:], 1.0)

            # scores / exp; psum slices come from the shared rotating pool
            for cc, (c0, csz) in enumerate(C_CHUNKS):
                ps = mmp.tile([128, PBLK], F32, tag="mm")
                for k in range(KC):
                    nc.tensor.matmul(ps[:csz, :O], cos_all[:, k, c0:c0 + csz],
                                     headsT[:, k, :], start=(k == 0),
                                     stop=False)
                for k in range(KC):
                    nc.tensor.matmul(ps[:csz, :O], sin_all[:, k, c0:c0 + csz],
                                     headsT[:, KC + k, :], start=False,
                                     stop=(k == KC - 1))
                nc.scalar.activation(expT[cc][:csz, :], ps[:csz, :O], ACT.Exp)
            # denominators as one [1, O] row: ones is the STATIONARY (1-column
            # LDWEIGHTS, ~free) and expT streams -- replaces 9 LDW+MM pairs
            # with 3.  The reciprocal row is broadcast over partitions and
            # folded into the weights, so psum drains become plain copies.
            ps = mmp.tile([128, PBLK], F32, tag="mm")
            for cc, (c0, csz) in enumerate(C_CHUNKS):
                nc.tensor.matmul(ps[0:1, 0:O], ones[:csz, 0:1],
                                 expT[cc][:csz, :],
                                 start=(cc == 0), stop=(cc == 2))
            recip_row = epool.tile([1, O], BF, tag="recip_row")
            with nc.allow_low_precision(reason="weights are bf16 anyway"):
                nc.vector.reciprocal(recip_row[:], ps[0:1, 0:O])
            recip_bc = epool.tile([128, O], BF, tag="recip_bc")
            nc.gpsimd.partition_broadcast(recip_bc[:], recip_row[:])
            wnorm = []
            for cc, (c0, csz) in enumerate(C_CHUNKS):
                wt = epool.tile([128, O], BF, tag=f"wn{cc}", name=f"wn{cc}")
                nc.vector.tensor_tensor(wt[:csz, :], expT[cc][:csz, :],
                                        recip_bc[:csz, :], ALU.mult)
                wnorm.append(wt)

            # ---- phase 2: two 8000-column halves, big DMAs ----
            dcount = 0
            for h in range(2):
                base = h * HALF
                xb = []
                for cc, (c0, csz) in enumerate(C_CHUNKS):
                    xt = xpool.tile([128, HALF], BF, tag=f"x{cc}",
                                    name=f"x{cc}")
                    nc.sync.dma_start(xt[:csz, :],
                                      xc_ap[c0:c0 + csz, base:base + HALF])
                    xb.append(xt)
                for oc, (o0, osz) in enumerate(C_CHUNKS):
                    ot = opool.tile([128, HALF], BF, tag=f"o{oc}",
                                    name=f"o{oc}")
                    for p0 in range(0, HALF, PBLK):
                        psz = min(PBLK, HALF - p0)
                        ph = mmp.tile([128, PBLK], F32, tag="mm")
                        for cc, (c0, csz) in enumerate(C_CHUNKS):
                            for s0 in range(0, psz, 512):
                                ssz = min(512, psz - s0)
                                nc.tensor.matmul(
                                    ph[:osz, s0:s0 + ssz],
                                    wnorm[cc][:csz, o0:o0 + osz],
                                    xb[cc][:csz, p0 + s0:p0 + s0 + ssz],
                                    start=(cc == 0), stop=(cc == 2))
                        # drain split across DVE+ACT so the psum tile frees
                        # in ~half the single-engine latency and both engines
                        # load evenly (alternate which engine takes the low
                        # half to decorrelate from other queue traffic)
                        hsz = psz // 2
                        lo = (ot[:osz, p0:p0 + hsz], ph[:osz, :hsz])
                        hi = (ot[:osz, p0 + hsz:p0 + psz], ph[:osz, hsz:psz])
                        a, b = (lo, hi) if dcount % 2 == 0 else (hi, lo)
                        dcount += 1
                        nc.vector.tensor_copy(a[0], a[1])
                        nc.scalar.activation(b[0], b[1], ACT.Copy)
                    nc.gpsimd.dma_start(out_ap[o0:o0 + osz, base:base + HALF],
                                        ot[:osz, :])

    nc.compile()
    return nc


def _tab_const():
    p = np.arange(128)
    cols = [(p & 31).astype(np.float32), np.ones(128, np.float32)]
    cols += [((k * 128 + p) >> 5).astype(np.float32) for k in range(KC)]
    return np.stack(cols, axis=1)


def _stage_heads(heads):
    hT = heads.T.astype(BF16)                     # [D, O]
    return np.ascontiguousarray(
        hT.reshape(2 * KC, 128, O).transpose(1, 0, 2))


def _stage_x(x_core):
    return np.ascontiguousarray(
        x_core.transpose(1, 0, 2).reshape(C, BLT).astype(BF16))


def get_nc(repeat=1):
    key = f"nc{repeat}"
    if key not in _cache:
        _cache[key] = _build(repeat)
    return _cache[key]


def make_in_maps(x, layout, heads):
    tab = _tab_const()
    ht = _stage_heads(heads.astype(np.float32))
    lay2 = np.ascontiguousarray(layout.astype(np.float32).T.reshape(1, 2 * C))
    return [
        {
            "xc": _stage_x(x[m * B_LOC:(m + 1) * B_LOC]),
            "lay2": lay2,
            "headsT": ht,
            "tab": tab,
        }
        for m in range(N_CORES)
    ]


def assemble_from_global(g):
    g = np.asarray(g).reshape(N_CORES, O, B_LOC, T)
    return np.ascontiguousarray(
        g.transpose(0, 2, 1, 3).reshape(B, O, T)).astype(np.float32)


def assemble_out(res_list):
    outs = []
    for m in range(N_CORES):
        o = np.asarray(res_list[m]["out"])
        o = o.reshape(O, B_LOC, T).transpose(1, 0, 2)
        outs.append(o.astype(np.float32))
    return np.concatenate(outs, axis=0)


def kernel(x, layout, heads):
    from concourse.bass_utils import run_bass_kernel_spmd

    assert x.shape == (B, C, T) and layout.shape == (C, 2)
    assert heads.shape == (O, D)
    nc = get_nc()
    in_maps = make_in_maps(x, layout, heads)
    res = run_bass_kernel_spmd(nc, in_maps, list(range(N_CORES)))
    return assemble_out(res.results)
